# revision 8
# baseline (speedup 1.0000x reference)
"""Multi-head causal attention (B=2, S=2048, D=1024, H=16) on 8 Trainium2
NeuronCores — optimized v2.

Sharding: data-parallel over the 2 batches x tensor-parallel over 4 head
groups (4 heads each).  Core c handles batch c//4, heads [4*(c%4), 4*(c%4)+4).
Host sums the 4 bf16 partial outputs per batch and adds the output bias.

v2 changes over the 237us baseline:
  * scores: the two heads of a partition-pair are computed CONCURRENTLY via
    row-tiled matmuls (tile_position (0,0)/(64,0), K=64 each) into two PSUM
    banks -> ~2x score matmul throughput.
  * causal handling: per-chunk column narrowing.  For the 4 diagonal key
    chunks of each query block only the live query columns are computed
    (N=512/384/256/128), and the remaining partial triangle is zeroed with
    one small [128,2,128] affine_select per chunk -> less ScalarE work, no
    full-width selects, no memsets.
  * softmax denominators: reciprocal_approx_fast (single custom DVE op,
    ~5x faster than the iterative-divide reciprocal that cost 53us).
  * phase overlap: q/k/v projection of block i+1 and the output projection
    of block i are emitted interleaved with attention of block i, so the
    TensorE projection work hides under the ACT-bound attention inner loop
    (and the PE stays HAM-warm).
  * x inputs are loaded with 4KB/partition lines and kept SBUF-resident;
    output partials are written back in bf16.
"""

import sys

sys.path.insert(0, "/opt/trn_rl_repo")

from collections import deque

import numpy as np

B, S, D, H = 2, 2048, 1024, 16
DK = D // H            # 64 head dim
NCORES = 8
NGROUPS = 4            # head groups (tensor parallel)
NH = H // NGROUPS      # 4 heads per core
DHL = NH * DK          # 256 local head dims per core
P = 128
DC = D // P            # 8 contraction chunks over D
HC = DHL // P          # 2 local head-dim chunks (= head pairs)
SB = 512               # query block
NSB = S // SB          # 4
SCK = S // P           # 16 key chunks

_CACHE = {}
DEBUG_DUMPS = False


def _build_nc(causal):
    import concourse.bass as bass
    import concourse.bacc as bacc
    import concourse.mybir as mybir
    import concourse.tile as tile
    from contextlib import ExitStack

    f32 = mybir.dt.float32
    bf16 = mybir.dt.bfloat16
    Exp = mybir.ActivationFunctionType.Exp
    is_ge = mybir.AluOpType.is_ge

    nc = bacc.Bacc(None, target_bir_lowering=False, debug=False)

    xq_d = nc.dram_tensor("xq_t", [D, S], bf16, kind="ExternalInput")
    xk_d = nc.dram_tensor("xk_t", [D, S], bf16, kind="ExternalInput")
    xv_d = nc.dram_tensor("xv_t", [D, S], bf16, kind="ExternalInput")
    wq_d = nc.dram_tensor("wq_a", [P, DC * DHL], bf16, kind="ExternalInput")
    wk_d = nc.dram_tensor("wk_a", [P, DC * DHL], bf16, kind="ExternalInput")
    wv_d = nc.dram_tensor("wv_a", [P, DC * DHL], bf16, kind="ExternalInput")
    wo_d = nc.dram_tensor("wo_a", [P, HC * D], bf16, kind="ExternalInput")
    bq_d = nc.dram_tensor("bq_a", [P, HC], f32, kind="ExternalInput")
    bk_d = nc.dram_tensor("bk_a", [P, HC], f32, kind="ExternalInput")
    bv_d = nc.dram_tensor("bv_a", [1, DHL], f32, kind="ExternalInput")
    out_d = nc.dram_tensor("out_t", [D, S], bf16, kind="ExternalOutput")
    if DEBUG_DUMPS:
        qdbg_d = nc.dram_tensor("qdbg", [P, HC * S], bf16,
                                kind="ExternalOutput")
        kdbg_d = nc.dram_tensor("kdbg", [P, NH * S], bf16,
                                kind="ExternalOutput")
        vdbg_d = nc.dram_tensor("vdbg", [P, SCK * NH * (DK + 1)], bf16,
                                kind="ExternalOutput")
        adbg_d = nc.dram_tensor("adbg", [P, HC * S], bf16,
                                kind="ExternalOutput")

    inv_sqrt_dk = 1.0 / float(np.sqrt(DK))

    with tile.TileContext(nc) as tc, ExitStack() as ctx:
        consts = ctx.enter_context(tc.tile_pool(name="consts", bufs=1))
        ex_pool = ctx.enter_context(tc.tile_pool(name="ex_pool", bufs=3))
        small = ctx.enter_context(tc.tile_pool(name="small", bufs=3))
        opool = ctx.enter_context(tc.tile_pool(name="opool", bufs=4))
        # proj groups and pv accumulators share one 4-slot pool: while a
        # pair's two pv banks sit in the (long) normalize chain, projection
        # fillers still have two banks to run in, so TensorE never starves
        proj_ps = ctx.enter_context(
            tc.tile_pool(name="proj_ps", bufs=4, space="PSUM"))
        sc_ps_pool = ctx.enter_context(
            tc.tile_pool(name="sc_ps", bufs=2, space="PSUM"))
        pv_ps_pool = proj_ps

        # --- resident tensors ---
        xq_sb = consts.tile([P, DC, S], bf16)
        xk_sb = consts.tile([P, DC, S], bf16)
        xv_sb = consts.tile([P, DC, S], bf16)
        wq_sb = consts.tile([P, DC, DHL], bf16)
        wk_sb = consts.tile([P, DC, DHL], bf16)
        wv_sb = consts.tile([P, DC, DHL], bf16)
        wo_sb = consts.tile([P, HC, D], bf16)
        bq_sb = consts.tile([P, HC], f32)
        bk_sb = consts.tile([P, HC], f32)
        bv_row = consts.tile([1, DHL], f32)
        bv_bc = consts.tile([P, DHL], f32)
        q_sb = consts.tile([P, HC, S], bf16)
        # K stored zero-padded per head: head h occupies its own 64
        # partitions, zeros elsewhere, so score matmuls run with K=128 and
        # the PE never switches tiling mode (mode switches drain the array)
        k_pad = consts.tile([P, NH, S], bf16)
        v_aug = consts.tile([P, SCK, NH, DK + 1], bf16)
        attn_sb = consts.tile([P, HC, S], bf16)

        nc.sync.dma_start(wq_sb[:], wq_d[:].rearrange("p (c h) -> p c h", c=DC))
        nc.sync.dma_start(wk_sb[:], wk_d[:].rearrange("p (c h) -> p c h", c=DC))
        nc.sync.dma_start(wv_sb[:], wv_d[:].rearrange("p (c h) -> p c h", c=DC))
        nc.sync.dma_start(wo_sb[:], wo_d[:].rearrange("p (c o) -> p c o", c=HC))
        nc.sync.dma_start(bq_sb[:], bq_d[:])
        nc.sync.dma_start(bk_sb[:], bk_d[:])
        nc.sync.dma_start(bv_row[:], bv_d[:])
        nc.gpsimd.partition_broadcast(bv_bc[:], bv_row[:])
        ones_f = consts.tile([P, SCK * NH], f32)
        nc.gpsimd.memset(ones_f[:], 1.0)
        nc.vector.tensor_copy(
            v_aug[:, :, :, DK],
            ones_f[:].rearrange("p (a b) -> p a b", a=SCK))

        nc.gpsimd.memset(k_pad[:], 0.0)

        bv_bc_r = bv_bc[:].rearrange("p (h e) -> p h e", h=NH)

        # x loads: 4KB/partition lines, SBUF resident, xq first
        for x_sb, x_d in ((xq_sb, xq_d), (xk_sb, xk_d), (xv_sb, xv_d)):
            for dc in range(DC):
                nc.sync.dma_start(
                    x_sb[:, dc, :],
                    x_d[:].rearrange("(c p) s -> p c s", p=P)[:, dc, :])

        # ---------- emission helpers ----------

        def emit_qk_group(sbi, w_sb, b_sb, t_sb, x_sb, hc):
            ss = slice(sbi * SB, (sbi + 1) * SB)
            ps = proj_ps.tile([P, SB], f32, name="proj", tag="ps")
            for dc in range(DC):
                nc.tensor.matmul(
                    ps[:], w_sb[:, dc, hc * P:(hc + 1) * P], x_sb[:, dc, ss],
                    start=(dc == 0), stop=(dc == DC - 1))
            if t_sb is None:  # k: scatter the two heads into padded slots
                for h2 in range(2):
                    po = h2 * DK
                    nc.vector.tensor_scalar_add(
                        k_pad[po:po + DK, 2 * hc + h2, ss], ps[po:po + DK, :],
                        b_sb[po:po + DK, hc:hc + 1])
            else:
                nc.vector.tensor_scalar_add(
                    t_sb[:, hc, ss], ps[:], b_sb[:, hc:hc + 1])

        def emit_v_group(sbi, scl):
            sc = sbi * (SB // P) + scl
            sl = slice(sbi * SB + scl * P, sbi * SB + (scl + 1) * P)
            ps = proj_ps.tile([P, DHL], f32, name="proj", tag="ps")
            for dc in range(DC):
                nc.tensor.matmul(
                    ps[:], xv_sb[:, dc, sl], wv_sb[:, dc, :],
                    start=(dc == 0), stop=(dc == DC - 1))
            nc.vector.tensor_add(
                v_aug[:, sc, :, 0:DK],
                ps[:].rearrange("p (h e) -> p h e", h=NH), bv_bc_r)

        def emit_out_group(sbi, ocp):
            ss = slice(sbi * SB, (sbi + 1) * SB)
            o_tile = opool.tile([P, 2, SB], bf16, name="ot", tag="ot")
            for j in range(2):
                oc = 2 * ocp + j
                ps = proj_ps.tile([P, SB], f32, name="proj", tag="ps")
                for hc in range(HC):
                    nc.tensor.matmul(
                        ps[:], wo_sb[:, hc, oc * P:(oc + 1) * P],
                        attn_sb[:, hc, ss], start=(hc == 0), stop=(hc == HC - 1))
                nc.scalar.copy(o_tile[:, j, :], ps[:])
            nc.sync.dma_start(
                out_d[:].rearrange("(c p) s -> p c s", p=P)
                [:, 2 * ocp:2 * ocp + 2, ss],
                o_tile[:])

        fillers = deque()

        def proj_block_fillers(sbi):
            fl = []
            for hc in range(HC):
                fl.append(lambda sbi=sbi, hc=hc: emit_qk_group(
                    sbi, wq_sb, bq_sb, q_sb, xq_sb, hc))
            for hc in range(HC):
                fl.append(lambda sbi=sbi, hc=hc: emit_qk_group(
                    sbi, wk_sb, bk_sb, None, xk_sb, hc))
            for scl in range(SB // P):
                fl.append(lambda sbi=sbi, scl=scl: emit_v_group(sbi, scl))
            return fl

        def out_block_fillers(sbi):
            return [lambda sbi=sbi, ocp=ocp: emit_out_group(sbi, ocp)
                    for ocp in range(DC // 2)]

        def pop_fillers(n):
            for _ in range(n):
                if fillers:
                    fillers.popleft()()

        # ---------- attention ----------

        def emit_pair_attention(hc, qb, pv_tiles):
            n_chunks = 4 * (qb + 1) if causal else SCK
            for c in range(n_chunks):
                rel = c - 4 * qb if causal else -1
                # diagonal chunks: only live query columns
                qoff = rel * P if rel >= 0 else 0
                n = SB - qoff
                qs = slice(qb * SB + qoff, qb * SB + qoff + n)

                sc_t = sc_ps_pool.tile([P, 2, SB], f32, name="sc", tag="sc")
                for h2 in range(2):
                    nc.tensor.matmul(
                        sc_t[:, h2, qoff:qoff + n],
                        k_pad[:, 2 * hc + h2, c * P:(c + 1) * P],
                        q_sb[:, hc, qs],
                        start=True, stop=True)
                ex = ex_pool.tile([P, 2, SB], bf16, name="ex", tag="ex")
                nc.scalar.activation(
                    ex[:, :, qoff:qoff + n], sc_t[:, :, qoff:qoff + n],
                    Exp, bias=0.0, scale=inv_sqrt_dk)
                if rel >= 0:
                    # zero the partial triangle: keep q_col - key_row >= 0
                    nc.gpsimd.affine_select(
                        ex[:, :, qoff:qoff + P], ex[:, :, qoff:qoff + P],
                        pattern=[[0, 2], [1, P]], compare_op=is_ge, fill=0.0,
                        base=0, channel_multiplier=-1)
                for h2 in range(2):
                    hl = 2 * hc + h2
                    nc.tensor.matmul(
                        pv_tiles[h2][:, qoff:qoff + n],
                        v_aug[:, c, hl, :],
                        ex[:, h2, qoff:qoff + n],
                        start=(c == 0), stop=(c == n_chunks - 1))
                pop_fillers(1)

        def emit_pair_normalize(hc, qb, pv_tiles):
            ss = slice(qb * SB, (qb + 1) * SB)
            for h2 in range(2):
                po = h2 * DK
                pv = pv_tiles[h2]
                pvs = small.tile([DK + 1, SB], f32, name="pvs", tag="pvs")
                nc.vector.tensor_copy(pvs[:], pv[:])
                recip = small.tile([1, SB], f32, name="recip", tag="recip")
                nc.vector.reciprocal(recip[:], pvs[DK:DK + 1, :])
                recip_bc = small.tile([DK, SB], f32, name="rbc", tag="rbc")
                nc.gpsimd.partition_broadcast(recip_bc[:], recip[:])
                nc.vector.tensor_mul(
                    attn_sb[po:po + DK, hc, ss], pvs[0:DK, :], recip_bc[:])

        # ---------- main emission ----------

        for fl in proj_block_fillers(0):
            fl()
        for qb in range(NSB):
            if qb + 1 < NSB:
                fillers.extend(proj_block_fillers(qb + 1))
            for hc in range(HC):
                pv_tiles = [
                    pv_ps_pool.tile([DK + 1, SB], f32, name="pv", tag="ps")
                    for _ in range(2)]
                emit_pair_attention(hc, qb, pv_tiles)
                emit_pair_normalize(hc, qb, pv_tiles)
            fillers.extend(out_block_fillers(qb))
        pop_fillers(len(fillers))
        if DEBUG_DUMPS:
            nc.sync.dma_start(
                qdbg_d[:].rearrange("p (c s) -> p c s", c=HC), q_sb[:])
            nc.sync.dma_start(
                kdbg_d[:].rearrange("p (c s) -> p c s", c=NH), k_pad[:])
            nc.sync.dma_start(
                vdbg_d[:].rearrange("p (a b c) -> p a b c", a=SCK, b=NH),
                v_aug[:])
            nc.sync.dma_start(
                adbg_d[:].rearrange("p (c s) -> p c s", c=HC), attn_sb[:])

    nc.compile()
    return nc


def _get_nc(causal):
    key = "causal" if causal else "dense"
    if key not in _CACHE:
        _CACHE[key] = _build_nc(causal)
    return _CACHE[key]


def _prep_core_inputs(Q, K, V, Wq, bq, Wk, bk, Wv, bv, Wo):
    """Build the 8 per-core input maps."""
    import ml_dtypes
    bf16 = ml_dtypes.bfloat16
    cc = np.ascontiguousarray
    in_maps = []
    for c in range(NCORES):
        b = c // NGROUPS
        g = c % NGROUPS
        hs, he = g * DHL, (g + 1) * DHL
        wq_a = cc(Wq[hs:he, :].T.reshape(DC, P, DHL).transpose(1, 0, 2)
                  .reshape(P, DC * DHL))
        wk_a = cc(Wk[hs:he, :].T.reshape(DC, P, DHL).transpose(1, 0, 2)
                  .reshape(P, DC * DHL))
        wv_a = cc(Wv[hs:he, :].T.reshape(DC, P, DHL).transpose(1, 0, 2)
                  .reshape(P, DC * DHL))
        wo_a = cc(Wo[:, hs:he].T.reshape(HC, P, D).transpose(1, 0, 2)
                  .reshape(P, HC * D))
        in_maps.append({
            "xq_t": cc(Q[b].T).astype(bf16), "xk_t": cc(K[b].T).astype(bf16),
            "xv_t": cc(V[b].T).astype(bf16),
            "wq_a": wq_a.astype(bf16), "wk_a": wk_a.astype(bf16),
            "wv_a": wv_a.astype(bf16), "wo_a": wo_a.astype(bf16),
            "bq_a": cc(bq[hs:he].reshape(HC, P).T),
            "bk_a": cc(bk[hs:he].reshape(HC, P).T),
            "bv_a": cc(bv[hs:he].reshape(1, DHL)),
        })
    return in_maps


def _classify_mask(mask):
    m = np.asarray(mask)
    if m.dtype != np.bool_:
        m = m.astype(bool)
    causal = np.tril(np.ones((S, S), dtype=bool))
    if all(np.array_equal(m[b, 0], causal) for b in range(m.shape[0])):
        return "causal"
    if m.all():
        return "dense"
    return "generic"


def _numpy_reference(Q, K, V, mask, Wq, bq, Wk, bk, Wv, bv, Wo, bo):
    out = np.empty((B, S, D), dtype=np.float32)
    for b in range(B):
        q = (Q[b] @ Wq.T + bq).reshape(S, H, DK).transpose(1, 0, 2)
        k = (K[b] @ Wk.T + bk).reshape(S, H, DK).transpose(1, 0, 2)
        v = (V[b] @ Wv.T + bv).reshape(S, H, DK).transpose(1, 0, 2)
        m = np.asarray(mask[b, 0], dtype=bool)
        acc = np.empty((H, S, DK), dtype=np.float32)
        for h in range(H):
            s = (q[h] @ k[h].T) / np.float32(np.sqrt(DK))
            s = np.where(m, s, np.float32(-1e9))
            s = s - s.max(axis=-1, keepdims=True)
            e = np.exp(s)
            p = e / e.sum(axis=-1, keepdims=True)
            acc[h] = p @ v[h]
        out[b] = acc.transpose(1, 0, 2).reshape(S, D) @ Wo.T + bo
    return out


def kernel(Q, K, V, mask, Wq, bq, Wk, bk, Wv, bv, Wo, bo,
           _profile=False, _trace_dir=None):
    from concourse.bass_utils import run_bass_kernel_spmd

    flavor = _classify_mask(mask)
    if flavor == "generic":
        return _numpy_reference(Q, K, V, mask, Wq, bq, Wk, bk, Wv, bv, Wo, bo)

    nc = _get_nc(flavor == "causal")
    in_maps = _prep_core_inputs(
        np.asarray(Q, np.float32), np.asarray(K, np.float32),
        np.asarray(V, np.float32), np.asarray(Wq, np.float32),
        np.asarray(bq, np.float32), np.asarray(Wk, np.float32),
        np.asarray(bk, np.float32), np.asarray(Wv, np.float32),
        np.asarray(bv, np.float32), np.asarray(Wo, np.float32))

    kwargs = {}
    if _profile:
        import importlib.util as _ilu
        _spec = _ilu.spec_from_file_location(
            "antenv.axon_hooks", "/opt/trn_rl_repo/antenv/axon_hooks.py")
        _mod = _ilu.module_from_spec(_spec)
        _spec.loader.exec_module(_mod)
        sys.modules["antenv.axon_hooks"] = _mod
        import concourse.bass_utils as _bu
        _bu.upload_artifacts = lambda d: d
        kwargs = dict(trace=True, trace_cores=[0])
        if _trace_dir is not None:
            kwargs["tmpdir"] = _trace_dir
    res = run_bass_kernel_spmd(nc, in_maps, core_ids=list(range(NCORES)),
                               **kwargs)

    out = np.empty((B, S, D), dtype=np.float32)
    bo32 = np.asarray(bo, np.float32)
    for b in range(B):
        acc = res.results[b * NGROUPS]["out_t"].astype(np.float32)
        for g in range(1, NGROUPS):
            acc = acc + res.results[b * NGROUPS + g]["out_t"].astype(np.float32)
        out[b] = acc.T + bo32
    if _profile:
        kernel._last_exec_time_ns = res.exec_time_ns
        kernel._last_results = res
    return out


# revision 10
# speedup vs baseline: 1.0703x; 1.0703x over previous
"""Multi-head causal attention (B=2, S=2048, D=1024, H=16) on 8 Trainium2
NeuronCores — optimized v2.

Sharding: data-parallel over the 2 batches x tensor-parallel over 4 head
groups (4 heads each).  Core c handles batch c//4, heads [4*(c%4), 4*(c%4)+4).
Host sums the 4 bf16 partial outputs per batch and adds the output bias.

v2 changes over the 237us baseline:
  * scores: the two heads of a partition-pair are computed CONCURRENTLY via
    row-tiled matmuls (tile_position (0,0)/(64,0), K=64 each) into two PSUM
    banks -> ~2x score matmul throughput.
  * causal handling: per-chunk column narrowing.  For the 4 diagonal key
    chunks of each query block only the live query columns are computed
    (N=512/384/256/128), and the remaining partial triangle is zeroed with
    one small [128,2,128] affine_select per chunk -> less ScalarE work, no
    full-width selects, no memsets.
  * softmax denominators: reciprocal_approx_fast (single custom DVE op,
    ~5x faster than the iterative-divide reciprocal that cost 53us).
  * phase overlap: q/k/v projection of block i+1 and the output projection
    of block i are emitted interleaved with attention of block i, so the
    TensorE projection work hides under the ACT-bound attention inner loop
    (and the PE stays HAM-warm).
  * x inputs are loaded with 4KB/partition lines and kept SBUF-resident;
    output partials are written back in bf16.
"""

import sys

sys.path.insert(0, "/opt/trn_rl_repo")

from collections import deque

import numpy as np

B, S, D, H = 2, 2048, 1024, 16
DK = D // H            # 64 head dim
NCORES = 8
NGROUPS = 4            # head groups (tensor parallel)
NH = H // NGROUPS      # 4 heads per core
DHL = NH * DK          # 256 local head dims per core
P = 128
DC = D // P            # 8 contraction chunks over D
HC = DHL // P          # 2 local head-dim chunks (= head pairs)
SB = 512               # query block
NSB = S // SB          # 4
SCK = S // P           # 16 key chunks

_CACHE = {}
DEBUG_DUMPS = False


def _build_nc(causal):
    import concourse.bass as bass
    import concourse.bacc as bacc
    import concourse.mybir as mybir
    import concourse.tile as tile
    from contextlib import ExitStack

    f32 = mybir.dt.float32
    bf16 = mybir.dt.bfloat16
    Exp = mybir.ActivationFunctionType.Exp
    is_ge = mybir.AluOpType.is_ge

    nc = bacc.Bacc(None, target_bir_lowering=False, debug=False)

    xq_d = nc.dram_tensor("xq_t", [D, S], bf16, kind="ExternalInput")
    xk_d = nc.dram_tensor("xk_t", [D, S], bf16, kind="ExternalInput")
    xv_d = nc.dram_tensor("xv_t", [D, S], bf16, kind="ExternalInput")
    wq_d = nc.dram_tensor("wq_a", [P, DC * DHL], bf16, kind="ExternalInput")
    wk_d = nc.dram_tensor("wk_a", [P, DC * DHL], bf16, kind="ExternalInput")
    wv_d = nc.dram_tensor("wv_a", [P, DC * DHL], bf16, kind="ExternalInput")
    wo_d = nc.dram_tensor("wo_a", [P, HC * D], bf16, kind="ExternalInput")
    bq_d = nc.dram_tensor("bq_a", [P, HC], f32, kind="ExternalInput")
    bk_d = nc.dram_tensor("bk_a", [P, HC], f32, kind="ExternalInput")
    bv_d = nc.dram_tensor("bv_a", [1, DHL], f32, kind="ExternalInput")
    out_d = nc.dram_tensor("out_t", [D, S], bf16, kind="ExternalOutput")
    if DEBUG_DUMPS:
        qdbg_d = nc.dram_tensor("qdbg", [P, HC * S], bf16,
                                kind="ExternalOutput")
        kdbg_d = nc.dram_tensor("kdbg", [P, NH * S], bf16,
                                kind="ExternalOutput")
        vdbg_d = nc.dram_tensor("vdbg", [P, SCK * NH * (DK + 1)], bf16,
                                kind="ExternalOutput")
        adbg_d = nc.dram_tensor("adbg", [P, HC * S], bf16,
                                kind="ExternalOutput")

    inv_sqrt_dk = 1.0 / float(np.sqrt(DK))

    with tile.TileContext(nc) as tc, ExitStack() as ctx:
        consts = ctx.enter_context(tc.tile_pool(name="consts", bufs=1))
        ex_pool = ctx.enter_context(tc.tile_pool(name="ex_pool", bufs=3))
        small = ctx.enter_context(tc.tile_pool(name="small", bufs=3))
        opool = ctx.enter_context(tc.tile_pool(name="opool", bufs=4))
        # proj groups and pv accumulators share one 4-slot pool: while a
        # pair's two pv banks sit in the (long) normalize chain, projection
        # fillers still have two banks to run in, so TensorE never starves
        proj_ps = ctx.enter_context(
            tc.tile_pool(name="proj_ps", bufs=4, space="PSUM"))
        sc_ps_pool = ctx.enter_context(
            tc.tile_pool(name="sc_ps", bufs=2, space="PSUM"))
        pv_ps_pool = proj_ps

        # --- resident tensors ---
        xq_sb = consts.tile([P, DC, S], bf16)
        xk_sb = consts.tile([P, DC, S], bf16)
        xv_sb = consts.tile([P, DC, S], bf16)
        wq_sb = consts.tile([P, DC, DHL], bf16)
        wk_sb = consts.tile([P, DC, DHL], bf16)
        wv_sb = consts.tile([P, DC, DHL], bf16)
        wo_sb = consts.tile([P, HC, D], bf16)
        bq_sb = consts.tile([P, HC], f32)
        bk_sb = consts.tile([P, HC], f32)
        bv_row = consts.tile([1, DHL], f32)
        bv_bc = consts.tile([P, DHL], f32)
        q_sb = consts.tile([P, HC, S], bf16)
        # K stored zero-padded per head: head h occupies its own 64
        # partitions, zeros elsewhere, so score matmuls run with K=128 and
        # the PE never switches tiling mode (mode switches drain the array)
        k_pad = consts.tile([P, NH, S], bf16)
        v_aug = consts.tile([P, SCK, NH, DK + 1], bf16)
        attn_sb = consts.tile([P, HC, S], bf16)

        nc.sync.dma_start(wq_sb[:], wq_d[:].rearrange("p (c h) -> p c h", c=DC))
        nc.sync.dma_start(wk_sb[:], wk_d[:].rearrange("p (c h) -> p c h", c=DC))
        nc.sync.dma_start(wv_sb[:], wv_d[:].rearrange("p (c h) -> p c h", c=DC))
        nc.sync.dma_start(wo_sb[:], wo_d[:].rearrange("p (c o) -> p c o", c=HC))
        nc.sync.dma_start(bq_sb[:], bq_d[:])
        nc.sync.dma_start(bk_sb[:], bk_d[:])
        nc.sync.dma_start(bv_row[:], bv_d[:])
        nc.gpsimd.partition_broadcast(bv_bc[:], bv_row[:])
        ones_f = consts.tile([P, SCK * NH], f32)
        nc.gpsimd.memset(ones_f[:], 1.0)
        nc.vector.tensor_copy(
            v_aug[:, :, :, DK],
            ones_f[:].rearrange("p (a b) -> p a b", a=SCK))

        nc.gpsimd.memset(k_pad[:], 0.0)

        bv_bc_r = bv_bc[:].rearrange("p (h e) -> p h e", h=NH)

        # x loads: block-major (one DMA per input per 512-block) so the
        # first projection + attention can start after ~3MB instead of 13MB
        for blk in range(NSB):
            bs = slice(blk * SB, (blk + 1) * SB)
            for x_sb, x_d in ((xq_sb, xq_d), (xk_sb, xk_d), (xv_sb, xv_d)):
                nc.sync.dma_start(
                    x_sb[:, :, bs],
                    x_d[:].rearrange("(c p) s -> p c s", p=P)[:, :, bs])

        # ---------- emission helpers ----------

        def emit_qk_group(sbi, w_sb, b_sb, t_sb, x_sb, hc):
            ss = slice(sbi * SB, (sbi + 1) * SB)
            ps = proj_ps.tile([P, SB], f32, name="proj", tag="ps")
            for dc in range(DC):
                nc.tensor.matmul(
                    ps[:], w_sb[:, dc, hc * P:(hc + 1) * P], x_sb[:, dc, ss],
                    start=(dc == 0), stop=(dc == DC - 1))
            if t_sb is None:  # k: scatter the two heads into padded slots
                for h2 in range(2):
                    po = h2 * DK
                    nc.vector.tensor_scalar_add(
                        k_pad[po:po + DK, 2 * hc + h2, ss], ps[po:po + DK, :],
                        b_sb[po:po + DK, hc:hc + 1])
            else:
                nc.vector.tensor_scalar_add(
                    t_sb[:, hc, ss], ps[:], b_sb[:, hc:hc + 1])

        def emit_v_group(sbi, scl):
            sc = sbi * (SB // P) + scl
            sl = slice(sbi * SB + scl * P, sbi * SB + (scl + 1) * P)
            ps = proj_ps.tile([P, DHL], f32, name="proj", tag="ps")
            for dc in range(DC):
                nc.tensor.matmul(
                    ps[:], xv_sb[:, dc, sl], wv_sb[:, dc, :],
                    start=(dc == 0), stop=(dc == DC - 1))
            nc.vector.tensor_add(
                v_aug[:, sc, :, 0:DK],
                ps[:].rearrange("p (h e) -> p h e", h=NH), bv_bc_r)

        def emit_out_group(sbi, ocp):
            ss = slice(sbi * SB, (sbi + 1) * SB)
            o_tile = opool.tile([P, 2, SB], bf16, name="ot", tag="ot")
            for j in range(2):
                oc = 2 * ocp + j
                ps = proj_ps.tile([P, SB], f32, name="proj", tag="ps")
                for hc in range(HC):
                    nc.tensor.matmul(
                        ps[:], wo_sb[:, hc, oc * P:(oc + 1) * P],
                        attn_sb[:, hc, ss], start=(hc == 0), stop=(hc == HC - 1))
                nc.scalar.copy(o_tile[:, j, :], ps[:])
            nc.sync.dma_start(
                out_d[:].rearrange("(c p) s -> p c s", p=P)
                [:, 2 * ocp:2 * ocp + 2, ss],
                o_tile[:])

        fillers = deque()

        def proj_block_fillers(sbi):
            fl = []
            for hc in range(HC):
                fl.append(lambda sbi=sbi, hc=hc: emit_qk_group(
                    sbi, wq_sb, bq_sb, q_sb, xq_sb, hc))
            for hc in range(HC):
                fl.append(lambda sbi=sbi, hc=hc: emit_qk_group(
                    sbi, wk_sb, bk_sb, None, xk_sb, hc))
            for scl in range(SB // P):
                fl.append(lambda sbi=sbi, scl=scl: emit_v_group(sbi, scl))
            return fl

        def out_block_fillers(sbi):
            return [lambda sbi=sbi, ocp=ocp: emit_out_group(sbi, ocp)
                    for ocp in range(DC // 2)]

        def pop_fillers(n):
            for _ in range(n):
                if fillers:
                    fillers.popleft()()

        # ---------- attention ----------

        def emit_pair_attention(hc, qb, pv_tiles):
            n_chunks = 4 * (qb + 1) if causal else SCK
            for c in range(n_chunks):
                rel = c - 4 * qb if causal else -1
                # diagonal chunks: only live query columns
                qoff = rel * P if rel >= 0 else 0
                n = SB - qoff
                qs = slice(qb * SB + qoff, qb * SB + qoff + n)

                sc_t = sc_ps_pool.tile([P, 2, SB], f32, name="sc", tag="sc")
                for h2 in range(2):
                    nc.tensor.matmul(
                        sc_t[:, h2, qoff:qoff + n],
                        k_pad[:, 2 * hc + h2, c * P:(c + 1) * P],
                        q_sb[:, hc, qs],
                        start=True, stop=True)
                ex = ex_pool.tile([P, 2, SB], bf16, name="ex", tag="ex")
                nc.scalar.activation(
                    ex[:, :, qoff:qoff + n], sc_t[:, :, qoff:qoff + n],
                    Exp, bias=0.0, scale=inv_sqrt_dk)
                if rel >= 0:
                    # zero the partial triangle: keep q_col - key_row >= 0
                    nc.gpsimd.affine_select(
                        ex[:, :, qoff:qoff + P], ex[:, :, qoff:qoff + P],
                        pattern=[[0, 2], [1, P]], compare_op=is_ge, fill=0.0,
                        base=0, channel_multiplier=-1)
                for h2 in range(2):
                    hl = 2 * hc + h2
                    nc.tensor.matmul(
                        pv_tiles[h2][:, qoff:qoff + n],
                        v_aug[:, c, hl, :],
                        ex[:, h2, qoff:qoff + n],
                        start=(c == 0), stop=(c == n_chunks - 1))
                pop_fillers(1)

        def emit_pair_normalize(hc, qb, pv_tiles):
            ss = slice(qb * SB, (qb + 1) * SB)
            # evacuate both pv banks back-to-back so the PSUM slots free
            # immediately (a reciprocal between the copies would hold the
            # second bank hostage in the DVE FIFO for 3.4us)
            pvss = []
            for h2 in range(2):
                pvs = small.tile([DK + 1, SB], f32, name="pvs", tag="pvs")
                nc.vector.tensor_copy(pvs[:], pv_tiles[h2][:])
                pvss.append(pvs)
            for h2 in range(2):
                po = h2 * DK
                recip = small.tile([1, SB], f32, name="recip", tag="recip")
                nc.vector.reciprocal(recip[:], pvss[h2][DK:DK + 1, :])
                recip_bc = small.tile([DK, SB], f32, name="rbc", tag="rbc")
                nc.gpsimd.partition_broadcast(recip_bc[:], recip[:])
                nc.vector.tensor_mul(
                    attn_sb[po:po + DK, hc, ss], pvss[h2][0:DK, :],
                    recip_bc[:])

        # ---------- main emission ----------

        for fl in proj_block_fillers(0):
            fl()
        for qb in range(NSB):
            if qb + 1 < NSB:
                fillers.extend(proj_block_fillers(qb + 1))
            for hc in range(HC):
                pv_tiles = [
                    pv_ps_pool.tile([DK + 1, SB], f32, name="pv", tag="ps")
                    for _ in range(2)]
                emit_pair_attention(hc, qb, pv_tiles)
                emit_pair_normalize(hc, qb, pv_tiles)
            fillers.extend(out_block_fillers(qb))
        pop_fillers(len(fillers))
        if DEBUG_DUMPS:
            nc.sync.dma_start(
                qdbg_d[:].rearrange("p (c s) -> p c s", c=HC), q_sb[:])
            nc.sync.dma_start(
                kdbg_d[:].rearrange("p (c s) -> p c s", c=NH), k_pad[:])
            nc.sync.dma_start(
                vdbg_d[:].rearrange("p (a b c) -> p a b c", a=SCK, b=NH),
                v_aug[:])
            nc.sync.dma_start(
                adbg_d[:].rearrange("p (c s) -> p c s", c=HC), attn_sb[:])

    nc.compile()
    return nc


def _get_nc(causal):
    key = "causal" if causal else "dense"
    if key not in _CACHE:
        _CACHE[key] = _build_nc(causal)
    return _CACHE[key]


def _prep_core_inputs(Q, K, V, Wq, bq, Wk, bk, Wv, bv, Wo):
    """Build the 8 per-core input maps."""
    import ml_dtypes
    bf16 = ml_dtypes.bfloat16
    cc = np.ascontiguousarray
    in_maps = []
    for c in range(NCORES):
        b = c // NGROUPS
        g = c % NGROUPS
        hs, he = g * DHL, (g + 1) * DHL
        wq_a = cc(Wq[hs:he, :].T.reshape(DC, P, DHL).transpose(1, 0, 2)
                  .reshape(P, DC * DHL))
        wk_a = cc(Wk[hs:he, :].T.reshape(DC, P, DHL).transpose(1, 0, 2)
                  .reshape(P, DC * DHL))
        wv_a = cc(Wv[hs:he, :].T.reshape(DC, P, DHL).transpose(1, 0, 2)
                  .reshape(P, DC * DHL))
        wo_a = cc(Wo[:, hs:he].T.reshape(HC, P, D).transpose(1, 0, 2)
                  .reshape(P, HC * D))
        in_maps.append({
            "xq_t": cc(Q[b].T).astype(bf16), "xk_t": cc(K[b].T).astype(bf16),
            "xv_t": cc(V[b].T).astype(bf16),
            "wq_a": wq_a.astype(bf16), "wk_a": wk_a.astype(bf16),
            "wv_a": wv_a.astype(bf16), "wo_a": wo_a.astype(bf16),
            "bq_a": cc(bq[hs:he].reshape(HC, P).T),
            "bk_a": cc(bk[hs:he].reshape(HC, P).T),
            "bv_a": cc(bv[hs:he].reshape(1, DHL)),
        })
    return in_maps


def _classify_mask(mask):
    m = np.asarray(mask)
    if m.dtype != np.bool_:
        m = m.astype(bool)
    causal = np.tril(np.ones((S, S), dtype=bool))
    if all(np.array_equal(m[b, 0], causal) for b in range(m.shape[0])):
        return "causal"
    if m.all():
        return "dense"
    return "generic"


def _numpy_reference(Q, K, V, mask, Wq, bq, Wk, bk, Wv, bv, Wo, bo):
    out = np.empty((B, S, D), dtype=np.float32)
    for b in range(B):
        q = (Q[b] @ Wq.T + bq).reshape(S, H, DK).transpose(1, 0, 2)
        k = (K[b] @ Wk.T + bk).reshape(S, H, DK).transpose(1, 0, 2)
        v = (V[b] @ Wv.T + bv).reshape(S, H, DK).transpose(1, 0, 2)
        m = np.asarray(mask[b, 0], dtype=bool)
        acc = np.empty((H, S, DK), dtype=np.float32)
        for h in range(H):
            s = (q[h] @ k[h].T) / np.float32(np.sqrt(DK))
            s = np.where(m, s, np.float32(-1e9))
            s = s - s.max(axis=-1, keepdims=True)
            e = np.exp(s)
            p = e / e.sum(axis=-1, keepdims=True)
            acc[h] = p @ v[h]
        out[b] = acc.transpose(1, 0, 2).reshape(S, D) @ Wo.T + bo
    return out


def kernel(Q, K, V, mask, Wq, bq, Wk, bk, Wv, bv, Wo, bo,
           _profile=False, _trace_dir=None):
    from concourse.bass_utils import run_bass_kernel_spmd

    flavor = _classify_mask(mask)
    if flavor == "generic":
        return _numpy_reference(Q, K, V, mask, Wq, bq, Wk, bk, Wv, bv, Wo, bo)

    nc = _get_nc(flavor == "causal")
    in_maps = _prep_core_inputs(
        np.asarray(Q, np.float32), np.asarray(K, np.float32),
        np.asarray(V, np.float32), np.asarray(Wq, np.float32),
        np.asarray(bq, np.float32), np.asarray(Wk, np.float32),
        np.asarray(bk, np.float32), np.asarray(Wv, np.float32),
        np.asarray(bv, np.float32), np.asarray(Wo, np.float32))

    kwargs = {}
    if _profile:
        import importlib.util as _ilu
        _spec = _ilu.spec_from_file_location(
            "antenv.axon_hooks", "/opt/trn_rl_repo/antenv/axon_hooks.py")
        _mod = _ilu.module_from_spec(_spec)
        _spec.loader.exec_module(_mod)
        sys.modules["antenv.axon_hooks"] = _mod
        import concourse.bass_utils as _bu
        _bu.upload_artifacts = lambda d: d
        kwargs = dict(trace=True, trace_cores=[0])
        if _trace_dir is not None:
            kwargs["tmpdir"] = _trace_dir
    res = run_bass_kernel_spmd(nc, in_maps, core_ids=list(range(NCORES)),
                               **kwargs)

    out = np.empty((B, S, D), dtype=np.float32)
    bo32 = np.asarray(bo, np.float32)
    for b in range(B):
        acc = res.results[b * NGROUPS]["out_t"].astype(np.float32)
        for g in range(1, NGROUPS):
            acc = acc + res.results[b * NGROUPS + g]["out_t"].astype(np.float32)
        out[b] = acc.T + bo32
    if _profile:
        kernel._last_exec_time_ns = res.exec_time_ns
        kernel._last_results = res
    return out


# revision 11
# speedup vs baseline: 1.2161x; 1.1362x over previous
"""Multi-head causal attention (B=2, S=2048, D=1024, H=16) on 8 Trainium2
NeuronCores — optimized v2.

Sharding: data-parallel over the 2 batches x tensor-parallel over 4 head
groups (4 heads each).  Core c handles batch c//4, heads [4*(c%4), 4*(c%4)+4).
Host sums the 4 bf16 partial outputs per batch and adds the output bias.

v2 changes over the 237us baseline:
  * scores: the two heads of a partition-pair are computed CONCURRENTLY via
    row-tiled matmuls (tile_position (0,0)/(64,0), K=64 each) into two PSUM
    banks -> ~2x score matmul throughput.
  * causal handling: per-chunk column narrowing.  For the 4 diagonal key
    chunks of each query block only the live query columns are computed
    (N=512/384/256/128), and the remaining partial triangle is zeroed with
    one small [128,2,128] affine_select per chunk -> less ScalarE work, no
    full-width selects, no memsets.
  * softmax denominators: reciprocal_approx_fast (single custom DVE op,
    ~5x faster than the iterative-divide reciprocal that cost 53us).
  * phase overlap: q/k/v projection of block i+1 and the output projection
    of block i are emitted interleaved with attention of block i, so the
    TensorE projection work hides under the ACT-bound attention inner loop
    (and the PE stays HAM-warm).
  * x inputs are loaded with 4KB/partition lines and kept SBUF-resident;
    output partials are written back in bf16.
"""

import sys

sys.path.insert(0, "/opt/trn_rl_repo")

from collections import deque

import numpy as np

B, S, D, H = 2, 2048, 1024, 16
DK = D // H            # 64 head dim
NCORES = 8
NGROUPS = 4            # head groups (tensor parallel)
NH = H // NGROUPS      # 4 heads per core
DHL = NH * DK          # 256 local head dims per core
P = 128
DC = D // P            # 8 contraction chunks over D
HC = DHL // P          # 2 local head-dim chunks (= head pairs)
SB = 512               # query block
NSB = S // SB          # 4
SCK = S // P           # 16 key chunks

_CACHE = {}
DEBUG_DUMPS = False


def _build_nc(causal):
    import concourse.bass as bass
    import concourse.bacc as bacc
    import concourse.mybir as mybir
    import concourse.tile as tile
    from contextlib import ExitStack

    f32 = mybir.dt.float32
    bf16 = mybir.dt.bfloat16
    Exp = mybir.ActivationFunctionType.Exp
    is_ge = mybir.AluOpType.is_ge

    nc = bacc.Bacc(None, target_bir_lowering=False, debug=False)

    xq_d = nc.dram_tensor("xq_t", [D, S], bf16, kind="ExternalInput")
    xk_d = nc.dram_tensor("xk_t", [D, S], bf16, kind="ExternalInput")
    xv_d = nc.dram_tensor("xv_t", [D, S], bf16, kind="ExternalInput")
    wq_d = nc.dram_tensor("wq_a", [P, DC * DHL], bf16, kind="ExternalInput")
    wk_d = nc.dram_tensor("wk_a", [P, DC * DHL], bf16, kind="ExternalInput")
    wv_d = nc.dram_tensor("wv_a", [P, DC * DHL], bf16, kind="ExternalInput")
    wo_d = nc.dram_tensor("wo_a", [P, HC * D], bf16, kind="ExternalInput")
    bq_d = nc.dram_tensor("bq_a", [P, HC], f32, kind="ExternalInput")
    bk_d = nc.dram_tensor("bk_a", [P, HC], f32, kind="ExternalInput")
    bv_d = nc.dram_tensor("bv_a", [1, DHL], f32, kind="ExternalInput")
    out_d = nc.dram_tensor("out_t", [D, S], bf16, kind="ExternalOutput")
    if DEBUG_DUMPS:
        qdbg_d = nc.dram_tensor("qdbg", [P, HC * S], bf16,
                                kind="ExternalOutput")
        kdbg_d = nc.dram_tensor("kdbg", [P, NH * S], bf16,
                                kind="ExternalOutput")
        vdbg_d = nc.dram_tensor("vdbg", [P, SCK * NH * (DK + 1)], bf16,
                                kind="ExternalOutput")
        adbg_d = nc.dram_tensor("adbg", [P, HC * S], bf16,
                                kind="ExternalOutput")

    inv_sqrt_dk = 1.0 / float(np.sqrt(DK))

    with tile.TileContext(nc) as tc, ExitStack() as ctx:
        consts = ctx.enter_context(tc.tile_pool(name="consts", bufs=1))
        ex_pool = ctx.enter_context(tc.tile_pool(name="ex_pool", bufs=3))
        small = ctx.enter_context(tc.tile_pool(name="small", bufs=3))
        opool = ctx.enter_context(tc.tile_pool(name="opool", bufs=4))
        # proj groups and pv accumulators share one 4-slot pool: while a
        # pair's two pv banks sit in the (long) normalize chain, projection
        # fillers still have two banks to run in, so TensorE never starves
        proj_ps = ctx.enter_context(
            tc.tile_pool(name="proj_ps", bufs=4, space="PSUM"))
        sc_ps_pool = ctx.enter_context(
            tc.tile_pool(name="sc_ps", bufs=2, space="PSUM"))
        pv_ps_pool = proj_ps

        # --- resident tensors ---
        xq_sb = consts.tile([P, DC, S], bf16)
        xk_sb = consts.tile([P, DC, S], bf16)
        xv_sb = consts.tile([P, DC, S], bf16)
        wq_sb = consts.tile([P, DC, DHL], bf16)
        wk_sb = consts.tile([P, DC, DHL], bf16)
        wv_sb = consts.tile([P, DC, DHL], bf16)
        wo_sb = consts.tile([P, HC, D], bf16)
        bq_sb = consts.tile([P, HC], f32)
        bk_sb = consts.tile([P, HC], f32)
        bv_row = consts.tile([1, DHL], f32)
        bv_bc = consts.tile([P, DHL], f32)
        q_sb = consts.tile([P, HC, S], bf16)
        # K stored zero-padded per head: head h occupies its own 64
        # partitions, zeros elsewhere, so score matmuls run with K=128 and
        # the PE never switches tiling mode (mode switches drain the array)
        k_pad = consts.tile([P, NH, S], bf16)
        v_aug = consts.tile([P, SCK, NH, DK + 1], bf16)
        attn_sb = consts.tile([P, HC, S], bf16)

        nc.sync.dma_start(wq_sb[:], wq_d[:].rearrange("p (c h) -> p c h", c=DC))
        nc.sync.dma_start(wk_sb[:], wk_d[:].rearrange("p (c h) -> p c h", c=DC))
        nc.sync.dma_start(wv_sb[:], wv_d[:].rearrange("p (c h) -> p c h", c=DC))
        nc.sync.dma_start(wo_sb[:], wo_d[:].rearrange("p (c o) -> p c o", c=HC))
        nc.sync.dma_start(bq_sb[:], bq_d[:])
        nc.sync.dma_start(bk_sb[:], bk_d[:])
        nc.sync.dma_start(bv_row[:], bv_d[:])
        nc.gpsimd.partition_broadcast(bv_bc[:], bv_row[:])
        ones_f = consts.tile([P, SCK * NH], f32)
        nc.gpsimd.memset(ones_f[:], 1.0)
        nc.vector.tensor_copy(
            v_aug[:, :, :, DK],
            ones_f[:].rearrange("p (a b) -> p a b", a=SCK))

        nc.gpsimd.memset(k_pad[:], 0.0)

        # constant triangle mask (keep q_col >= key_row) for diagonal blocks,
        # applied as a DVE multiply so GpSimd stays off the exp->PV path
        tri_mask = consts.tile([P, 2, P], bf16)
        nc.gpsimd.memset(tri_mask[:], 1.0)
        nc.gpsimd.affine_select(
            tri_mask[:], tri_mask[:],
            pattern=[[0, 2], [1, P]], compare_op=is_ge, fill=0.0,
            base=0, channel_multiplier=-1)

        bv_bc_r = bv_bc[:].rearrange("p (h e) -> p h e", h=NH)

        # x loads: block-major (one DMA per input per 512-block) so the
        # first projection + attention can start after ~3MB instead of 13MB
        for blk in range(NSB):
            bs = slice(blk * SB, (blk + 1) * SB)
            for x_sb, x_d in ((xq_sb, xq_d), (xk_sb, xk_d), (xv_sb, xv_d)):
                nc.sync.dma_start(
                    x_sb[:, :, bs],
                    x_d[:].rearrange("(c p) s -> p c s", p=P)[:, :, bs])

        # ---------- emission helpers ----------

        def emit_qk_group(sbi, w_sb, b_sb, t_sb, x_sb, hc):
            ss = slice(sbi * SB, (sbi + 1) * SB)
            ps = proj_ps.tile([P, SB], f32, name="proj", tag="ps")
            for dc in range(DC):
                nc.tensor.matmul(
                    ps[:], w_sb[:, dc, hc * P:(hc + 1) * P], x_sb[:, dc, ss],
                    start=(dc == 0), stop=(dc == DC - 1))
            if t_sb is None:  # k: scatter the two heads into padded slots
                for h2 in range(2):
                    po = h2 * DK
                    nc.vector.tensor_scalar_add(
                        k_pad[po:po + DK, 2 * hc + h2, ss], ps[po:po + DK, :],
                        b_sb[po:po + DK, hc:hc + 1])
            else:
                nc.vector.tensor_scalar_add(
                    t_sb[:, hc, ss], ps[:], b_sb[:, hc:hc + 1])

        def emit_v_group(sbi, scl):
            sc = sbi * (SB // P) + scl
            sl = slice(sbi * SB + scl * P, sbi * SB + (scl + 1) * P)
            ps = proj_ps.tile([P, DHL], f32, name="proj", tag="ps")
            for dc in range(DC):
                nc.tensor.matmul(
                    ps[:], xv_sb[:, dc, sl], wv_sb[:, dc, :],
                    start=(dc == 0), stop=(dc == DC - 1))
            nc.vector.tensor_add(
                v_aug[:, sc, :, 0:DK],
                ps[:].rearrange("p (h e) -> p h e", h=NH), bv_bc_r)

        def emit_out_group(sbi, ocp):
            ss = slice(sbi * SB, (sbi + 1) * SB)
            o_tile = opool.tile([P, 2, SB], bf16, name="ot", tag="ot")
            for j in range(2):
                oc = 2 * ocp + j
                ps = proj_ps.tile([P, SB], f32, name="proj", tag="ps")
                for hc in range(HC):
                    nc.tensor.matmul(
                        ps[:], wo_sb[:, hc, oc * P:(oc + 1) * P],
                        attn_sb[:, hc, ss], start=(hc == 0), stop=(hc == HC - 1))
                nc.scalar.copy(o_tile[:, j, :], ps[:])
            nc.sync.dma_start(
                out_d[:].rearrange("(c p) s -> p c s", p=P)
                [:, 2 * ocp:2 * ocp + 2, ss],
                o_tile[:])

        fillers = deque()

        def proj_block_fillers(sbi):
            fl = []
            for hc in range(HC):
                fl.append(lambda sbi=sbi, hc=hc: emit_qk_group(
                    sbi, wq_sb, bq_sb, q_sb, xq_sb, hc))
            for hc in range(HC):
                fl.append(lambda sbi=sbi, hc=hc: emit_qk_group(
                    sbi, wk_sb, bk_sb, None, xk_sb, hc))
            for scl in range(SB // P):
                fl.append(lambda sbi=sbi, scl=scl: emit_v_group(sbi, scl))
            return fl

        def out_block_fillers(sbi):
            return [lambda sbi=sbi, ocp=ocp: emit_out_group(sbi, ocp)
                    for ocp in range(DC // 2)]

        def pop_fillers(n):
            for _ in range(n):
                if fillers:
                    fillers.popleft()()

        # ---------- attention ----------

        def emit_pair_attention(hc, qb, pv_tiles):
            n_chunks = 4 * (qb + 1) if causal else SCK
            for c in range(n_chunks):
                rel = c - 4 * qb if causal else -1
                # diagonal chunks: only live query columns
                qoff = rel * P if rel >= 0 else 0
                n = SB - qoff
                qs = slice(qb * SB + qoff, qb * SB + qoff + n)

                sc_t = sc_ps_pool.tile([P, 2, SB], f32, name="sc", tag="sc")
                for h2 in range(2):
                    nc.tensor.matmul(
                        sc_t[:, h2, qoff:qoff + n],
                        k_pad[:, 2 * hc + h2, c * P:(c + 1) * P],
                        q_sb[:, hc, qs],
                        start=True, stop=True)
                ex = ex_pool.tile([P, 2, SB], bf16, name="ex", tag="ex")
                nc.scalar.activation(
                    ex[:, :, qoff:qoff + n], sc_t[:, :, qoff:qoff + n],
                    Exp, bias=0.0, scale=inv_sqrt_dk)
                if rel >= 0:
                    # zero the partial triangle: keep q_col - key_row >= 0
                    nc.vector.tensor_mul(
                        ex[:, :, qoff:qoff + P], ex[:, :, qoff:qoff + P],
                        tri_mask[:])
                for h2 in range(2):
                    hl = 2 * hc + h2
                    nc.tensor.matmul(
                        pv_tiles[h2][:, qoff:qoff + n],
                        v_aug[:, c, hl, :],
                        ex[:, h2, qoff:qoff + n],
                        start=(c == 0), stop=(c == n_chunks - 1))
                pop_fillers(1)

        def emit_pair_normalize(hc, qb, pv_tiles):
            ss = slice(qb * SB, (qb + 1) * SB)
            # evacuate both pv banks on ScalarE (fast PSUM port, keeps the
            # DVE queue clear); the PSUM slots free right after
            pvss = []
            for h2 in range(2):
                pvs = small.tile([DK + 1, SB], f32, name="pvs", tag="pvs")
                nc.scalar.copy(pvs[:], pv_tiles[h2][:])
                pvss.append(pvs)
            # both denominators onto partitions 0/32 of one tile via tiny
            # SBUF->SBUF DMA hops, then ONE reciprocal call: DVE reciprocal
            # cost is per-lane, so [33,512] serves both heads in 3.4us
            den2 = small.tile([33, SB], f32, name="den2", tag="den2")
            nc.gpsimd.memset(den2[:], 1.0)
            nc.sync.dma_start(den2[0:1, :], pvss[0][DK:DK + 1, :])
            nc.sync.dma_start(den2[32:33, :], pvss[1][DK:DK + 1, :])
            rec2 = small.tile([33, SB], f32, name="rec2", tag="rec2")
            nc.vector.reciprocal(rec2[:], den2[:])
            recb = small.tile([1, SB], f32, name="recb", tag="recb")
            nc.sync.dma_start(recb[:], rec2[32:33, :])
            for h2, rsrc in ((0, rec2[0:1, :]), (1, recb[:])):
                po = h2 * DK
                recip_bc = small.tile([DK, SB], f32, name="rbc", tag="rbc")
                nc.gpsimd.partition_broadcast(recip_bc[:], rsrc)
                nc.vector.tensor_mul(
                    attn_sb[po:po + DK, hc, ss], pvss[h2][0:DK, :],
                    recip_bc[:])

        # ---------- main emission ----------

        for fl in proj_block_fillers(0):
            fl()
        for qb in range(NSB):
            if qb + 1 < NSB:
                fillers.extend(proj_block_fillers(qb + 1))
            for hc in range(HC):
                pv_tiles = [
                    pv_ps_pool.tile([DK + 1, SB], f32, name="pv", tag="ps")
                    for _ in range(2)]
                emit_pair_attention(hc, qb, pv_tiles)
                emit_pair_normalize(hc, qb, pv_tiles)
            fillers.extend(out_block_fillers(qb))
        pop_fillers(len(fillers))
        if DEBUG_DUMPS:
            nc.sync.dma_start(
                qdbg_d[:].rearrange("p (c s) -> p c s", c=HC), q_sb[:])
            nc.sync.dma_start(
                kdbg_d[:].rearrange("p (c s) -> p c s", c=NH), k_pad[:])
            nc.sync.dma_start(
                vdbg_d[:].rearrange("p (a b c) -> p a b c", a=SCK, b=NH),
                v_aug[:])
            nc.sync.dma_start(
                adbg_d[:].rearrange("p (c s) -> p c s", c=HC), attn_sb[:])

    nc.compile()
    return nc


def _get_nc(causal):
    key = "causal" if causal else "dense"
    if key not in _CACHE:
        _CACHE[key] = _build_nc(causal)
    return _CACHE[key]


def _prep_core_inputs(Q, K, V, Wq, bq, Wk, bk, Wv, bv, Wo):
    """Build the 8 per-core input maps."""
    import ml_dtypes
    bf16 = ml_dtypes.bfloat16
    cc = np.ascontiguousarray
    in_maps = []
    for c in range(NCORES):
        b = c // NGROUPS
        g = c % NGROUPS
        hs, he = g * DHL, (g + 1) * DHL
        wq_a = cc(Wq[hs:he, :].T.reshape(DC, P, DHL).transpose(1, 0, 2)
                  .reshape(P, DC * DHL))
        wk_a = cc(Wk[hs:he, :].T.reshape(DC, P, DHL).transpose(1, 0, 2)
                  .reshape(P, DC * DHL))
        wv_a = cc(Wv[hs:he, :].T.reshape(DC, P, DHL).transpose(1, 0, 2)
                  .reshape(P, DC * DHL))
        wo_a = cc(Wo[:, hs:he].T.reshape(HC, P, D).transpose(1, 0, 2)
                  .reshape(P, HC * D))
        in_maps.append({
            "xq_t": cc(Q[b].T).astype(bf16), "xk_t": cc(K[b].T).astype(bf16),
            "xv_t": cc(V[b].T).astype(bf16),
            "wq_a": wq_a.astype(bf16), "wk_a": wk_a.astype(bf16),
            "wv_a": wv_a.astype(bf16), "wo_a": wo_a.astype(bf16),
            "bq_a": cc(bq[hs:he].reshape(HC, P).T),
            "bk_a": cc(bk[hs:he].reshape(HC, P).T),
            "bv_a": cc(bv[hs:he].reshape(1, DHL)),
        })
    return in_maps


def _classify_mask(mask):
    m = np.asarray(mask)
    if m.dtype != np.bool_:
        m = m.astype(bool)
    causal = np.tril(np.ones((S, S), dtype=bool))
    if all(np.array_equal(m[b, 0], causal) for b in range(m.shape[0])):
        return "causal"
    if m.all():
        return "dense"
    return "generic"


def _numpy_reference(Q, K, V, mask, Wq, bq, Wk, bk, Wv, bv, Wo, bo):
    out = np.empty((B, S, D), dtype=np.float32)
    for b in range(B):
        q = (Q[b] @ Wq.T + bq).reshape(S, H, DK).transpose(1, 0, 2)
        k = (K[b] @ Wk.T + bk).reshape(S, H, DK).transpose(1, 0, 2)
        v = (V[b] @ Wv.T + bv).reshape(S, H, DK).transpose(1, 0, 2)
        m = np.asarray(mask[b, 0], dtype=bool)
        acc = np.empty((H, S, DK), dtype=np.float32)
        for h in range(H):
            s = (q[h] @ k[h].T) / np.float32(np.sqrt(DK))
            s = np.where(m, s, np.float32(-1e9))
            s = s - s.max(axis=-1, keepdims=True)
            e = np.exp(s)
            p = e / e.sum(axis=-1, keepdims=True)
            acc[h] = p @ v[h]
        out[b] = acc.transpose(1, 0, 2).reshape(S, D) @ Wo.T + bo
    return out


def kernel(Q, K, V, mask, Wq, bq, Wk, bk, Wv, bv, Wo, bo,
           _profile=False, _trace_dir=None):
    from concourse.bass_utils import run_bass_kernel_spmd

    flavor = _classify_mask(mask)
    if flavor == "generic":
        return _numpy_reference(Q, K, V, mask, Wq, bq, Wk, bk, Wv, bv, Wo, bo)

    nc = _get_nc(flavor == "causal")
    in_maps = _prep_core_inputs(
        np.asarray(Q, np.float32), np.asarray(K, np.float32),
        np.asarray(V, np.float32), np.asarray(Wq, np.float32),
        np.asarray(bq, np.float32), np.asarray(Wk, np.float32),
        np.asarray(bk, np.float32), np.asarray(Wv, np.float32),
        np.asarray(bv, np.float32), np.asarray(Wo, np.float32))

    kwargs = {}
    if _profile:
        import importlib.util as _ilu
        _spec = _ilu.spec_from_file_location(
            "antenv.axon_hooks", "/opt/trn_rl_repo/antenv/axon_hooks.py")
        _mod = _ilu.module_from_spec(_spec)
        _spec.loader.exec_module(_mod)
        sys.modules["antenv.axon_hooks"] = _mod
        import concourse.bass_utils as _bu
        _bu.upload_artifacts = lambda d: d
        kwargs = dict(trace=True, trace_cores=[0])
        if _trace_dir is not None:
            kwargs["tmpdir"] = _trace_dir
    res = run_bass_kernel_spmd(nc, in_maps, core_ids=list(range(NCORES)),
                               **kwargs)

    out = np.empty((B, S, D), dtype=np.float32)
    bo32 = np.asarray(bo, np.float32)
    for b in range(B):
        acc = res.results[b * NGROUPS]["out_t"].astype(np.float32)
        for g in range(1, NGROUPS):
            acc = acc + res.results[b * NGROUPS + g]["out_t"].astype(np.float32)
        out[b] = acc.T + bo32
    if _profile:
        kernel._last_exec_time_ns = res.exec_time_ns
        kernel._last_results = res
    return out
